# revision 1
# baseline (speedup 1.0000x reference)
"""Deformable Conv2d (modulated, v2) on 8 Trainium2 NeuronCores via Bass.

Sharding: data-parallel over (batch=4) x (image half=2) = 8 shards.
Each core: offset/mask convs for its 2048 output pixels (9 accumulating
matmuls over a zero-padded input window) -> PE-transpose to pixel-major ->
bilinear weights psi + gather row indices on DVE -> indirect-DMA gather of
x-pair rows (bf16, row-major padded image in DRAM) -> per-partition-scalar
modulation by psi -> identity-matmul transpose accumulating the 4 bilinear
neighbors into val[c, p] -> 9-tap main conv as accumulating matmuls -> out.
"""
import sys

if "/opt/trn_rl_repo" not in sys.path:
    sys.path.insert(0, "/opt/trn_rl_repo")

import numpy as np
import ml_dtypes

import concourse.bass as bass
import concourse.tile as tile
import concourse.mybir as mybir
from concourse.bass_utils import run_bass_kernel_spmd
from concourse.masks import make_identity

F32 = mybir.dt.float32
BF16 = mybir.dt.bfloat16
I32 = mybir.dt.int32
ALU = mybir.AluOpType
ACTF = mybir.ActivationFunctionType

B, C, O, H, W = 4, 128, 128, 64, 64
K2 = 9
HALVES = 2
N_CORES = B * HALVES
PIX = H * W // HALVES          # 2048 pixels per core
NPT = PIX // 128               # 16 pixel-tiles per core
HROWS = H // HALVES            # 32 image rows per core
WP = W + 2                     # padded row width
HPAD = HROWS + 2               # 34 padded rows staged per core
XT_ROWS = H * W + 8            # 1 zero row + 4096 + tail pad
OMC = 27                       # 18 offset + 9 mask channels
KN = K2 * NPT                  # 144


def _split_fat_waits(nc, max_waits=1):
    """This walrus build rejects instructions carrying more than ~1 sync wait;
    move excess waits onto preceding same-engine NoOps (engine stalls at each,
    so semantics are preserved)."""
    for f in nc.m.functions:
        for bb in f.blocks:
            newlist = []
            for ins in bb.instructions:
                si = ins.sync_info
                if si and si.on_wait and len(si.on_wait) > max_waits:
                    waits = list(si.on_wait)
                    extra, keep = waits[:-max_waits], waits[-max_waits:]
                    for i in range(0, len(extra), max_waits):
                        chunk = extra[i:i + max_waits]
                        nop = mybir.InstNoOp(
                            name=nc.get_next_instruction_name(),
                            text_hint="split_wait",
                        )
                        nop.engine = ins.engine
                        nop.sync_info = mybir.SyncInfo(on_wait=chunk, on_update=[])
                        newlist.append(nop)
                    si.on_wait = keep
                newlist.append(ins)
            bb.instructions[:] = newlist


def build_nc(reps=1, debug=False):
    nc = bass.Bass()
    tc = tile.TileContext(nc)

    # ---- DRAM I/O (per-core tensors; program is SPMD-identical) ----
    x_pad = nc.dram_tensor("x_pad", [C, HPAD * WP], BF16, kind="ExternalInput")
    xt_pad = nc.dram_tensor("xt_pad", [XT_ROWS, C], BF16, kind="ExternalInput")
    w_main = nc.dram_tensor("w_main", [C, K2 * O], BF16, kind="ExternalInput")
    w_om = nc.dram_tensor("w_om", [C, K2 * OMC], BF16, kind="ExternalInput")
    b_om = nc.dram_tensor("b_om", [OMC, 1], F32, kind="ExternalInput")
    base_y = nc.dram_tensor("base_y", [128, KN], F32, kind="ExternalInput")
    base_x = nc.dram_tensor("base_x", [128, KN], F32, kind="ExternalInput")
    out_d = nc.dram_tensor("out", [O, PIX], F32, kind="ExternalOutput")
    if debug:
        dbg_om = nc.dram_tensor("dbg_om", [OMC, PIX], F32, kind="ExternalOutput")
        dbg_omT = nc.dram_tensor("dbg_omT", [128, NPT * OMC], F32, kind="ExternalOutput")
        dbg_psi = nc.dram_tensor("dbg_psi", [128, 4 * KN], F32, kind="ExternalOutput")
        dbg_q01 = nc.dram_tensor("dbg_q01", [128, KN * 2], I32, kind="ExternalOutput")
        dbg_val = nc.dram_tensor("dbg_val", [C, K2 * PIX], BF16, kind="ExternalOutput")
        dbg_gk = nc.dram_tensor("dbg_gk", [128, 2 * NPT * 2 * C], BF16, kind="ExternalOutput")

    TT = nc.vector.tensor_tensor
    TS = nc.vector.tensor_scalar

    with tc:
        with tc.tile_pool(name="persist", bufs=1) as pp, \
             tc.tile_pool(name="work", bufs=2) as wp, \
             tc.tile_pool(name="gbuf", bufs=3) as gp, \
             tc.tile_pool(name="gs", bufs=4) as gsp, \
             tc.tile_pool(name="psA", bufs=2, space="PSUM") as psA, \
             tc.tile_pool(name="psB", bufs=2, space="PSUM") as psB, \
             tc.tile_pool(name="psO", bufs=1, space="PSUM") as psO:

            # ---- persistent SBUF loads ----
            xp = pp.tile([C, HPAD * WP], BF16)
            nc.sync.dma_start(xp[:], x_pad[:])
            wm = pp.tile([C, K2 * O], BF16)
            nc.sync.dma_start(wm[:], w_main[:])
            wo = pp.tile([C, K2 * OMC], BF16)
            nc.sync.dma_start(wo[:], w_om[:])
            bo = pp.tile([OMC, 1], F32)
            nc.sync.dma_start(bo[:], b_om[:])
            bY = pp.tile([128, KN], F32)
            nc.sync.dma_start(bY[:], base_y[:])
            bX = pp.tile([128, KN], F32)
            nc.sync.dma_start(bX[:], base_x[:])
            ident = pp.tile([128, 128], BF16)
            make_identity(nc, ident[:])
            identf = pp.tile([OMC, OMC], F32)
            make_identity(nc, identf[:])

            # persistent buffers (reused across reps)
            om = pp.tile([OMC, PIX], F32)
            omT = pp.tile([128, NPT * OMC], F32)
            val = pp.tile([C, K2 * PIX], BF16)
            q01 = pp.tile([128, KN * 2], I32)
            out_sb = pp.tile([O, PIX], F32)
            psi = [pp.tile([128, KN], F32, name=f"psi{n}") for n in range(4)]

            for _rep in range(reps):
                # ============ Phase 1: offset/mask convs ============
                for g in range(4):             # 512-pixel groups = 8 rows
                    p_om = psA.tile([OMC, 512], F32, name="p_om")
                    for k in range(K2):
                        ki, kj = divmod(k, 3)
                        off = (8 * g + ki) * WP
                        rhs = xp[:, off:off + 8 * WP].rearrange(
                            "c (r w) -> c r w", r=8, w=WP)[:, :, kj:kj + W]
                        nc.tensor.matmul(
                            p_om[:], wo[:, k * OMC:(k + 1) * OMC], rhs,
                            start=(k == 0), stop=(k == K2 - 1))
                    TS(out=om[:, g * 512:(g + 1) * 512], in0=p_om[:],
                       scalar1=bo[:, 0:1], scalar2=None, op0=ALU.add)

                # ============ Phase 2: transpose om to pixel-major ============
                for pth in range(NPT // 2):
                    p_omT = psB.tile([128, 2 * OMC], F32, name="pvt", tag="pvt")
                    for h2 in range(2):
                        pt = 2 * pth + h2
                        nc.tensor.transpose(
                            p_omT[:, h2 * OMC:(h2 + 1) * OMC],
                            om[:, pt * 128:(pt + 1) * 128],
                            identf[:])
                    nc.vector.tensor_copy(
                        omT[:, 2 * pth * OMC:(2 * pth + 2) * OMC], p_omT[:])

                # ============ Phase 3: psi weights + gather indices ===========
                dy = wp.tile([128, KN], F32, name="dy")
                dx = wp.tile([128, KN], F32, name="dx")
                mk = wp.tile([128, KN], F32, name="mk")
                src = omT[:].rearrange("p (t j) -> p j t", j=OMC)
                for k in range(K2):
                    nc.vector.tensor_copy(dy[:, k * NPT:(k + 1) * NPT],
                                          src[:, 2 * k, :])
                    nc.vector.tensor_copy(dx[:, k * NPT:(k + 1) * NPT],
                                          src[:, 2 * k + 1, :])
                    nc.vector.tensor_copy(mk[:, k * NPT:(k + 1) * NPT],
                                          src[:, 18 + k, :])
                nc.scalar.activation(mk[:], mk[:], ACTF.Sigmoid)

                py = wp.tile([128, KN], F32, name="py")
                px = wp.tile([128, KN], F32, name="px")
                TT(out=py[:], in0=bY[:], in1=dy[:], op=ALU.add)
                TT(out=px[:], in0=bX[:], in1=dx[:], op=ALU.add)

                # floor via +16 / trunc-cast / -16 (py >= -3.x always)
                yi = wp.tile([128, KN], I32, name="yi")
                xi = wp.tile([128, KN], I32, name="xi")
                y0f = wp.tile([128, KN], F32, name="y0f")
                x0f = wp.tile([128, KN], F32, name="x0f")
                TS(out=y0f[:], in0=py[:], scalar1=15.5, scalar2=None, op0=ALU.add)
                nc.vector.tensor_copy(yi[:], y0f[:])
                nc.vector.tensor_copy(y0f[:], yi[:])
                TS(out=y0f[:], in0=y0f[:], scalar1=-16.0, scalar2=None, op0=ALU.add)
                TS(out=x0f[:], in0=px[:], scalar1=15.5, scalar2=None, op0=ALU.add)
                nc.vector.tensor_copy(xi[:], x0f[:])
                nc.vector.tensor_copy(x0f[:], xi[:])
                TS(out=x0f[:], in0=x0f[:], scalar1=-16.0, scalar2=None, op0=ALU.add)

                wy = wp.tile([128, KN], F32, name="wy")
                wx = wp.tile([128, KN], F32, name="wx")
                TT(out=wy[:], in0=py[:], in1=y0f[:], op=ALU.subtract)
                TT(out=wx[:], in0=px[:], in1=x0f[:], op=ALU.subtract)

                # clamps + validity (valid <=> clamp is identity)
                y0c = wp.tile([128, KN], F32, name="y0c")
                y1c = wp.tile([128, KN], F32, name="y1c")
                x0c = wp.tile([128, KN], F32, name="x0c")
                t0 = wp.tile([128, KN], F32, name="t0")
                vy0 = wp.tile([128, KN], F32, name="vy0")
                vy1 = wp.tile([128, KN], F32, name="vy1")
                vx0 = wp.tile([128, KN], F32, name="vx0")
                vx1 = wp.tile([128, KN], F32, name="vx1")
                TS(out=y0c[:], in0=y0f[:], scalar1=0.0, scalar2=63.0,
                   op0=ALU.max, op1=ALU.min)
                TT(out=vy0[:], in0=y0c[:], in1=y0f[:], op=ALU.is_equal)
                TS(out=y1c[:], in0=y0f[:], scalar1=-1.0, scalar2=62.0,
                   op0=ALU.max, op1=ALU.min)
                TT(out=vy1[:], in0=y1c[:], in1=y0f[:], op=ALU.is_equal)
                TS(out=x0c[:], in0=x0f[:], scalar1=-1.0, scalar2=63.0,
                   op0=ALU.max, op1=ALU.min)
                TS(out=t0[:], in0=x0f[:], scalar1=0.0, scalar2=63.0,
                   op0=ALU.max, op1=ALU.min)
                TT(out=vx0[:], in0=t0[:], in1=x0f[:], op=ALU.is_equal)
                TS(out=t0[:], in0=x0f[:], scalar1=-1.0, scalar2=62.0,
                   op0=ALU.max, op1=ALU.min)
                TT(out=vx1[:], in0=t0[:], in1=x0f[:], op=ALU.is_equal)

                # psi terms
                u0 = wp.tile([128, KN], F32, name="u0")
                v0 = wp.tile([128, KN], F32, name="v0")
                a0 = wp.tile([128, KN], F32, name="a0")
                a1 = wp.tile([128, KN], F32, name="a1")
                c0 = wp.tile([128, KN], F32, name="c0")
                c1 = wp.tile([128, KN], F32, name="c1")
                TS(out=u0[:], in0=wy[:], scalar1=-1.0, scalar2=1.0,
                   op0=ALU.mult, op1=ALU.add)
                TS(out=v0[:], in0=wx[:], scalar1=-1.0, scalar2=1.0,
                   op0=ALU.mult, op1=ALU.add)
                TT(out=a0[:], in0=mk[:], in1=u0[:], op=ALU.mult)
                TT(out=a0[:], in0=a0[:], in1=vy0[:], op=ALU.mult)
                TT(out=a1[:], in0=mk[:], in1=wy[:], op=ALU.mult)
                TT(out=a1[:], in0=a1[:], in1=vy1[:], op=ALU.mult)
                TT(out=c0[:], in0=v0[:], in1=vx0[:], op=ALU.mult)
                TT(out=c1[:], in0=wx[:], in1=vx1[:], op=ALU.mult)
                TT(out=psi[0][:], in0=a0[:], in1=c0[:], op=ALU.mult)
                TT(out=psi[1][:], in0=a0[:], in1=c1[:], op=ALU.mult)
                TT(out=psi[2][:], in0=a1[:], in1=c0[:], op=ALU.mult)
                TT(out=psi[3][:], in0=a1[:], in1=c1[:], op=ALU.mult)

                # gather row indices into the 1-shifted padded image:
                # q0 = y0c*64 + x0c + 1 ; q1 = (y1c+1)*64 + x0c + 1
                q0 = wp.tile([128, KN], F32, name="q0")
                q1 = wp.tile([128, KN], F32, name="q1")
                TS(out=q0[:], in0=y0c[:], scalar1=64.0, scalar2=1.0,
                   op0=ALU.mult, op1=ALU.add)
                TT(out=q0[:], in0=q0[:], in1=x0c[:], op=ALU.add)
                TS(out=q1[:], in0=y1c[:], scalar1=64.0, scalar2=65.0,
                   op0=ALU.mult, op1=ALU.add)
                TT(out=q1[:], in0=q1[:], in1=x0c[:], op=ALU.add)
                q01v = q01[:].rearrange("p (k t y) -> p k t y", k=K2, y=2)
                for k in range(K2):
                    nc.vector.tensor_copy(q01v[:, k, :, 0], q0[:, k * NPT:(k + 1) * NPT])
                    nc.vector.tensor_copy(q01v[:, k, :, 1], q1[:, k * NPT:(k + 1) * NPT])

                # ======== Phase 4: gather, modulate, transpose, main conv ========
                p_out = psO.tile([O, PIX], F32, name="p_out")
                if debug:
                    dbg_gk_sb = pp.tile([128, 2 * NPT, 2 * C], BF16, name="dbg_gk_sb")
                for k in range(K2):
                    gk = gp.tile([128, 2 * NPT, 2 * C], BF16, name="gk")
                    for blk in range(2 * NPT):
                        cb = k * 2 * NPT + blk
                        nc.gpsimd.indirect_dma_start(
                            out=gk[:, blk, :], out_offset=None, in_=xt_pad[:],
                            in_offset=bass.IndirectOffsetOnAxis(
                                ap=q01[:, cb:cb + 1], axis=0),
                        )
                    if debug and k == 0:
                        nc.vector.tensor_copy(dbg_gk_sb[:], gk[:])
                    for pt in range(NPT):
                        gs = gsp.tile([128, 4 * C], BF16, name="gs")
                        col = k * NPT + pt
                        for yn in range(2):
                            for xs in range(2):
                                n = 2 * yn + xs
                                TS(out=gs[:, n * C:(n + 1) * C],
                                   in0=gk[:, 2 * pt + yn, xs * C:(xs + 1) * C],
                                   scalar1=psi[n][:, col:col + 1],
                                   scalar2=None, op0=ALU.mult)
                        p_vt = psB.tile([128, 128], F32, name="pvt", tag="pvt")
                        for n in range(4):
                            nc.tensor.matmul(
                                p_vt[:], gs[:, n * C:(n + 1) * C], ident[:],
                                start=(n == 0), stop=(n == 3))
                        nc.scalar.copy(
                            val[:, col * 128:(col + 1) * 128], p_vt[:])
                    # main conv contribution of tap k
                    for g in range(4):
                        nc.tensor.matmul(
                            p_out[:, g * 512:(g + 1) * 512],
                            wm[:, k * O:(k + 1) * O],
                            val[:, k * PIX + g * 512:k * PIX + (g + 1) * 512],
                            start=(k == 0), stop=(k == K2 - 1))

                if debug:
                    nc.sync.dma_start(dbg_om[:], om[:])
                    nc.sync.dma_start(dbg_omT[:], omT[:])
                    for n in range(4):
                        nc.sync.dma_start(dbg_psi[:, n * KN:(n + 1) * KN], psi[n][:])
                    nc.sync.dma_start(dbg_q01[:], q01[:])
                    nc.sync.dma_start(dbg_val[:], val[:])
                    nc.sync.dma_start(dbg_gk[:], dbg_gk_sb[:].rearrange("p a b -> p (a b)"))
                # ================= Phase 5: write out =================
                for g in range(4):
                    nc.vector.tensor_copy(out_sb[:, g * 512:(g + 1) * 512],
                                          p_out[:, g * 512:(g + 1) * 512])
                nc.sync.dma_start(out_d[:], out_sb[:])

    _split_fat_waits(nc)
    nc.finalize()
    return nc


# ---------------- host-side data prep ----------------

def prep_in_maps(x, org_w, offset_w, offset_b, mask_w, mask_b):
    x = np.asarray(x, dtype=np.float32)
    org_w = np.asarray(org_w, dtype=np.float32)
    offset_w = np.asarray(offset_w, dtype=np.float32)
    offset_b = np.asarray(offset_b, dtype=np.float32)
    mask_w = np.asarray(mask_w, dtype=np.float32)
    mask_b = np.asarray(mask_b, dtype=np.float32)

    wm = org_w.reshape(O, C, K2).transpose(1, 2, 0)          # [C, K2, O]
    wm = np.ascontiguousarray(wm.reshape(C, K2 * O)).astype(ml_dtypes.bfloat16)
    wo = np.concatenate([offset_w.reshape(18, C, K2),
                         mask_w.reshape(9, C, K2)], axis=0)  # [27, C, K2]
    wo = wo.transpose(1, 2, 0)                               # [C, K2, 27]
    wo = np.ascontiguousarray(wo.reshape(C, K2 * OMC)).astype(ml_dtypes.bfloat16)
    bom = np.concatenate([offset_b, mask_b]).reshape(OMC, 1).astype(np.float32)

    in_maps = []
    for b in range(B):
        xb = x[b].reshape(C, H, W)
        xpadf = np.zeros((C, H + 2, WP), np.float32)
        xpadf[:, 1:H + 1, 1:W + 1] = xb
        xt = np.zeros((XT_ROWS, C), np.float32)
        xt[1:H * W + 1] = xb.reshape(C, H * W).T
        xt = xt.astype(ml_dtypes.bfloat16)
        for h in range(HALVES):
            # padded rows [32h, 32h+34) of the full padded image
            xpad_core = np.ascontiguousarray(
                xpadf[:, 32 * h:32 * h + HPAD, :].reshape(C, HPAD * WP)
            ).astype(ml_dtypes.bfloat16)
            p = h * PIX + np.arange(PIX)
            r = np.arange(PIX) % 128
            pt = np.arange(PIX) // 128
            yy = (p // W).astype(np.float32)
            xx = (p % W).astype(np.float32)
            bY = np.zeros((128, KN), np.float32)
            bX = np.zeros((128, KN), np.float32)
            for k in range(K2):
                ki, kj = divmod(k, 3)
                bY[r, k * NPT + pt] = yy - 1 + ki
                bX[r, k * NPT + pt] = xx - 1 + kj
            in_maps.append({
                "x_pad": xpad_core, "xt_pad": xt, "w_main": wm, "w_om": wo,
                "b_om": bom, "base_y": bY, "base_x": bX,
            })
    return in_maps


_NC_CACHE = {}


def _get_nc(reps=1):
    if reps not in _NC_CACHE:
        _NC_CACHE[reps] = build_nc(reps)
    return _NC_CACHE[reps]


def assemble(results):
    out = np.zeros((B, O, H, W), np.float32)
    for core in range(N_CORES):
        b, h = divmod(core, HALVES)
        o = np.asarray(results[core]["out"])
        out[b, :, h * HROWS:(h + 1) * HROWS, :] = o.reshape(O, HROWS, W)
    return out


def kernel(x, org_w, offset_w, offset_b, mask_w, mask_b):
    nc = _get_nc(1)
    in_maps = prep_in_maps(x, org_w, offset_w, offset_b, mask_w, mask_b)
    res = run_bass_kernel_spmd(nc, in_maps, core_ids=list(range(N_CORES)))
    return assemble(res.results)



# revision 2
# speedup vs baseline: 12.6929x; 12.6929x over previous
"""Deformable Conv2d (modulated, v2) on 8 Trainium2 NeuronCores via Bass.

v2 design, instruction-count-minimized (~275/rep vs ~2000 in v1):
- offset/mask conv -> om PSUM [96, 2048] (dy rows 0-8, dx 32-40, mk 64-72 via
  zero-padded weights; 32-aligned bases for legal DVE slicing)
- psi/anchor pipeline on [9, 2048] tap-major tiles (~25 DVE ops total)
- single-anchor gather: host-precomputed xq[r] = 4 bilinear neighbors of
  padded-image position r (1KB rows); zero padding absorbs all validity
  masking; 16 indirect DMAs per tap ([128,1] offsets)
- xbar DMA-transpose for psi/q/val tap->pixel / pixel->channel relayouts
- modulation: one stride-0-broadcast TT per tap; 4-neighbor presum via one
  strided tensor_reduce; 4 accumulating conv matmuls per tap
Sharding: data-parallel (batch=4) x (image half=2) = 8 cores.
"""
import sys

if "/opt/trn_rl_repo" not in sys.path:
    sys.path.insert(0, "/opt/trn_rl_repo")

import numpy as np
import ml_dtypes

import concourse.bass as bass
import concourse.tile as tile
import concourse.mybir as mybir
from concourse.bass_utils import run_bass_kernel_spmd

F32 = mybir.dt.float32
BF16 = mybir.dt.bfloat16
I32 = mybir.dt.int32
I16 = mybir.dt.int16
ALU = mybir.AluOpType
ACTF = mybir.ActivationFunctionType

B, C, O, H, W = 4, 128, 128, 64, 64
K2 = 9
HALVES = 2
N_CORES = B * HALVES
PIX = H * W // HALVES          # 2048 pixels per core
NPT = PIX // 128               # 16 pixel-tiles per core
HROWS = H // HALVES            # 32 image rows per core
WP = W + 2                     # padded row width (phase-1 conv)
HPAD = HROWS + 2               # 34 padded rows staged per core
PAD = 4                        # gather-table padding
PW = W + 2 * PAD               # 72
NQ = PW * PW                   # 5184 anchor rows


def _split_fat_waits(nc, max_waits=1):
    """This walrus build rejects instructions carrying more than ~1 sync wait;
    move excess waits onto preceding same-engine NoOps."""
    for f in nc.m.functions:
        for bb in f.blocks:
            newlist = []
            for ins in bb.instructions:
                si = ins.sync_info
                if si and si.on_wait and len(si.on_wait) > max_waits:
                    waits = list(si.on_wait)
                    extra, keep = waits[:-max_waits], waits[-max_waits:]
                    for i in range(0, len(extra), max_waits):
                        chunk = extra[i:i + max_waits]
                        nop = mybir.InstNoOp(
                            name=nc.get_next_instruction_name(),
                            text_hint="split_wait",
                        )
                        nop.engine = ins.engine
                        nop.sync_info = mybir.SyncInfo(on_wait=chunk, on_update=[])
                        newlist.append(nop)
                    si.on_wait = keep
                newlist.append(ins)
            bb.instructions[:] = newlist


def build_nc(reps=1):
    nc = bass.Bass()
    tc = tile.TileContext(nc)

    x_pad = nc.dram_tensor("x_pad", [C, HPAD * WP], BF16, kind="ExternalInput")
    xq_d = nc.dram_tensor("xq", [NQ, 4 * C], BF16, kind="ExternalInput")
    wo_d = nc.dram_tensor("wo96", [C, K2 * 96], BF16, kind="ExternalInput")
    wm_d = nc.dram_tensor("w_main", [C, K2 * O], BF16, kind="ExternalInput")
    by_d = nc.dram_tensor("bY16", [K2, PIX], F32, kind="ExternalInput")
    bx_d = nc.dram_tensor("bX16", [K2, PIX], F32, kind="ExternalInput")
    mb_d = nc.dram_tensor("mb", [K2, 1], F32, kind="ExternalInput")
    out_d = nc.dram_tensor("out", [O, PIX], F32, kind="ExternalOutput")

    TT = nc.vector.tensor_tensor
    TS = nc.vector.tensor_scalar

    with tc:
        with tc.tile_pool(name="persist", bufs=1) as pp, \
             tc.tile_pool(name="work", bufs=1) as wp, \
             tc.tile_pool(name="gbuf", bufs=2) as gp, \
             tc.tile_pool(name="vbuf", bufs=2) as vp, \
             tc.tile_pool(name="psA", bufs=1, space="PSUM") as psA, \
             tc.tile_pool(name="psO", bufs=1, space="PSUM") as psO:

            # ---- persistent loads ----
            xp = pp.tile([C, HPAD * WP], BF16)
            nc.sync.dma_start(xp[:], x_pad[:])
            wo = pp.tile([C, K2 * 96], BF16)
            nc.sync.dma_start(wo[:], wo_d[:])
            wm = pp.tile([C, K2 * O], BF16)
            nc.sync.dma_start(wm[:], wm_d[:])
            bY = pp.tile([K2, PIX], F32)
            nc.sync.dma_start(bY[:], by_d[:])
            bX = pp.tile([K2, PIX], F32)
            nc.sync.dma_start(bX[:], bx_d[:])
            mb = pp.tile([K2, 1], F32)
            nc.sync.dma_start(mb[:], mb_d[:])

            q16 = pp.tile([16, PIX], I16)
            nc.vector.memset(q16[:], 0)
            psi_all = pp.tile([128, PIX], BF16)
            nc.vector.memset(psi_all[:], 0.0)
            out_sb = pp.tile([O, PIX], F32)

            for _rep in range(reps):
                # ============ Phase 1: offset/mask conv ============
                om_ps = psA.tile([96, PIX], F32, name="om_ps")
                for g in range(4):
                    for k in range(K2):
                        ki, kj = divmod(k, 3)
                        off = (8 * g + ki) * WP
                        rhs = xp[:, off:off + 8 * WP].rearrange(
                            "c (r w) -> c r w", r=8, w=WP)[:, :, kj:kj + W]
                        nc.tensor.matmul(
                            om_ps[:, g * 512:(g + 1) * 512],
                            wo[:, k * 96:(k + 1) * 96], rhs,
                            start=(k == 0), stop=(k == K2 - 1))
                om_sb = wp.tile([96, PIX], F32, name="om_sb")
                nc.scalar.copy(om_sb[:], om_ps[:])
                dxt = wp.tile([K2, PIX], F32, name="dxt")
                nc.sync.dma_start(dxt[:], om_sb[32:32 + K2, :])
                mkt = wp.tile([K2, PIX], F32, name="mkt")
                nc.sync.dma_start(mkt[:], om_sb[64:64 + K2, :])

                # ============ Phase 2: psi + anchors (tap-major) ============
                nc.scalar.activation(mkt[:], mkt[:], ACTF.Sigmoid,
                                     bias=mb[:, 0:1])
                py = wp.tile([K2, PIX], F32, name="py")
                TT(out=py[:], in0=om_sb[0:K2, :], in1=bY[:], op=ALU.add)
                px = wp.tile([K2, PIX], F32, name="px")
                TT(out=px[:], in0=dxt[:], in1=bX[:], op=ALU.add)

                yi = wp.tile([K2, PIX], I32, name="yi")
                nc.vector.tensor_copy(yi[:], py[:])       # rounds
                y0p = wp.tile([K2, PIX], F32, name="y0p")
                nc.vector.tensor_copy(y0p[:], yi[:])
                xi = wp.tile([K2, PIX], I32, name="xi")
                nc.vector.tensor_copy(xi[:], px[:])
                x0p = wp.tile([K2, PIX], F32, name="x0p")
                nc.vector.tensor_copy(x0p[:], xi[:])

                # py/px become wy'/wx' = w - 0.5
                TT(out=py[:], in0=py[:], in1=y0p[:], op=ALU.subtract)
                TT(out=px[:], in0=px[:], in1=x0p[:], op=ALU.subtract)
                wy = wp.tile([K2, PIX], F32, name="wy")
                TS(out=wy[:], in0=py[:], scalar1=0.5, scalar2=None, op0=ALU.add)
                wx = wp.tile([K2, PIX], F32, name="wx")
                TS(out=wx[:], in0=px[:], scalar1=0.5, scalar2=None, op0=ALU.add)
                # py/px become u0/v0 = 1 - w
                TS(out=py[:], in0=py[:], scalar1=-1.0, scalar2=0.5,
                   op0=ALU.mult, op1=ALU.add)
                TS(out=px[:], in0=px[:], scalar1=-1.0, scalar2=0.5,
                   op0=ALU.mult, op1=ALU.add)

                # anchor clamps: y0 in [-4, 66] <=> y0p in [12, 82]
                TS(out=y0p[:], in0=y0p[:], scalar1=12.0, scalar2=82.0,
                   op0=ALU.max, op1=ALU.min)
                TS(out=x0p[:], in0=x0p[:], scalar1=12.0, scalar2=82.0,
                   op0=ALU.max, op1=ALU.min)

                # psi products (bf16, written to 32-aligned partition blocks)
                a0 = wp.tile([K2, PIX], F32, name="a0")
                TT(out=a0[:], in0=mkt[:], in1=py[:], op=ALU.mult)
                TT(out=mkt[:], in0=mkt[:], in1=wy[:], op=ALU.mult)  # a1
                TT(out=psi_all[0:K2, :], in0=a0[:], in1=px[:], op=ALU.mult)
                TT(out=psi_all[32:32 + K2, :], in0=a0[:], in1=wx[:], op=ALU.mult)
                TT(out=psi_all[64:64 + K2, :], in0=mkt[:], in1=px[:], op=ALU.mult)
                TT(out=psi_all[96:96 + K2, :], in0=mkt[:], in1=wx[:], op=ALU.mult)

                # anchor q = (y0c+4)*72 + x0c+4, biased coords -> -876
                TS(out=y0p[:], in0=y0p[:], scalar1=72.0, scalar2=-876.0,
                   op0=ALU.mult, op1=ALU.add)
                TT(out=y0p[:], in0=y0p[:], in1=x0p[:], op=ALU.add)
                nc.vector.tensor_copy(q16[0:K2, :], y0p[:])

                # tap-major -> pixel-major via xbar
                qT16 = wp.tile([128, NPT * 16], I16, name="qT16")
                nc.sync.dma_start(
                    qT16[:].rearrange("p (t r) -> p t r", t=NPT),
                    q16[:], transpose=True)
                qTi = wp.tile([128, NPT * 16], I32, name="qTi")
                nc.vector.tensor_copy(qTi[:], qT16[:])
                psiT = wp.tile([128, NPT * 128], BF16, name="psiT")
                nc.sync.dma_start(
                    psiT[:].rearrange("p (t r) -> p t r", t=NPT),
                    psi_all[:], transpose=True)

                # ======== Phase 3/4: gather, modulate, presum, conv ========
                p_out = psO.tile([O, PIX], F32, name="p_out")
                pT = psiT[:]
                psi_pstride = pT.ap[0][0]
                for k in range(K2):
                    gq = gp.tile([128, NPT, 4 * C], BF16, name="gq")
                    for t in range(NPT):
                        nc.gpsimd.indirect_dma_start(
                            out=gq[:, t, :], out_offset=None, in_=xq_d[:],
                            in_offset=bass.IndirectOffsetOnAxis(
                                ap=qTi[:, t * 16 + k:t * 16 + k + 1], axis=0),
                        )
                    # modulate in place: gq *= psi (stride-0 bcast over c)
                    in1 = bass.AP(pT.tensor, pT.offset + k,
                                  [[psi_pstride, 128], [128, NPT],
                                   [32, 4], [0, C]])
                    TT(out=gq[:].rearrange("p t (n c) -> p t n c", n=4),
                       in0=gq[:].rearrange("p t (n c) -> p t n c", n=4),
                       in1=in1, op=ALU.mult)
                    # presum 4 neighbors
                    val = vp.tile([128, NPT, C], BF16, name="val")
                    with nc.allow_low_precision("4-term bilinear presum"):
                        nc.vector.tensor_reduce(
                            val[:],
                            gq[:].rearrange("p t (n c) -> p t c n", n=4),
                            axis=mybir.AxisListType.X, op=ALU.add)
                    # pixel-major -> channel-major
                    valT = vp.tile([128, NPT, 128], BF16, name="valT")
                    nc.sync.dma_start(valT[:],
                                      val[:].rearrange("p t c -> p (t c)"),
                                      transpose=True)
                    vT = valT[:].rearrange("c t p -> c (t p)")
                    for g in range(4):
                        nc.tensor.matmul(
                            p_out[:, g * 512:(g + 1) * 512],
                            wm[:, k * O:(k + 1) * O],
                            vT[:, g * 512:(g + 1) * 512],
                            start=(k == 0), stop=(k == K2 - 1))

                # ================= Phase 5: write out =================
                nc.scalar.copy(out_sb[:], p_out[:])
                nc.sync.dma_start(out_d[:], out_sb[:])

    _split_fat_waits(nc)
    nc.finalize()
    return nc


# ---------------- host-side data prep ----------------

def prep_in_maps(x, org_w, offset_w, offset_b, mask_w, mask_b):
    x = np.asarray(x, dtype=np.float32)
    org_w = np.asarray(org_w, dtype=np.float32)
    offset_w = np.asarray(offset_w, dtype=np.float32)
    offset_b = np.asarray(offset_b, dtype=np.float32)
    mask_w = np.asarray(mask_w, dtype=np.float32)
    mask_b = np.asarray(mask_b, dtype=np.float32)

    wm = org_w.reshape(O, C, K2).transpose(1, 2, 0)          # [C, K2, O]
    wm = np.ascontiguousarray(wm.reshape(C, K2 * O)).astype(ml_dtypes.bfloat16)

    # wo96: per-tap [C, 96]: cols 0-8 dy_j, 32-40 dx_j, 64-72 mk_j
    wo96 = np.zeros((C, K2, 96), np.float32)
    ow = offset_w.reshape(2 * K2, C, K2)                     # [ch, C, tap]
    mw = mask_w.reshape(K2, C, K2)
    for j in range(K2):
        wo96[:, :, j] = ow[2 * j]                            # dy_j  [C, tap]
        wo96[:, :, 32 + j] = ow[2 * j + 1]                   # dx_j
        wo96[:, :, 64 + j] = mw[j]                           # mk_j
    wo96 = np.ascontiguousarray(
        wo96.reshape(C, K2 * 96)).astype(ml_dtypes.bfloat16)

    mb = mask_b.reshape(K2, 1).astype(np.float32)

    in_maps = []
    for b in range(B):
        xb = x[b].reshape(C, H, W)
        xpadf = np.zeros((C, H + 2, WP), np.float32)
        xpadf[:, 1:H + 1, 1:W + 1] = xb

        # anchor table: padded 72x72 image, 4 neighbors per row
        xpad72 = np.zeros((PW * PW + PW + 2, C), np.float32)
        grid = xpad72[:PW * PW].reshape(PW, PW, C)
        grid[PAD:PAD + H, PAD:PAD + W] = xb.transpose(1, 2, 0)
        xq = np.concatenate(
            [xpad72[0:NQ], xpad72[1:NQ + 1],
             xpad72[PW:NQ + PW], xpad72[PW + 1:NQ + PW + 1]],
            axis=1).astype(ml_dtypes.bfloat16)               # [NQ, 4*C]

        for h in range(HALVES):
            xpad_core = np.ascontiguousarray(
                xpadf[:, HROWS * h:HROWS * h + HPAD, :].reshape(C, HPAD * WP)
            ).astype(ml_dtypes.bfloat16)
            p = np.arange(PIX)
            oy = (h * HROWS + p // W).astype(np.float32)
            ox = (p % W).astype(np.float32)
            bY = np.zeros((K2, PIX), np.float32)
            bX = np.zeros((K2, PIX), np.float32)
            for j in range(K2):
                ki, kj = divmod(j, 3)
                bY[j] = oy + ki - 1 + offset_b[2 * j] + 15.5
                bX[j] = ox + kj - 1 + offset_b[2 * j + 1] + 15.5
            in_maps.append({
                "x_pad": xpad_core, "xq": xq, "wo96": wo96, "w_main": wm,
                "bY16": bY, "bX16": bX, "mb": mb,
            })
    return in_maps


_NC_CACHE = {}


def _get_nc(reps=1):
    if reps not in _NC_CACHE:
        _NC_CACHE[reps] = build_nc(reps)
    return _NC_CACHE[reps]


def assemble(results):
    out = np.zeros((B, O, H, W), np.float32)
    for core in range(N_CORES):
        b, h = divmod(core, HALVES)
        o = np.asarray(results[core]["out"])
        out[b, :, h * HROWS:(h + 1) * HROWS, :] = o.reshape(O, HROWS, W)
    return out


def kernel(x, org_w, offset_w, offset_b, mask_w, mask_b):
    nc = _get_nc(1)
    in_maps = prep_in_maps(x, org_w, offset_w, offset_b, mask_w, mask_b)
    res = run_bass_kernel_spmd(nc, in_maps, core_ids=list(range(N_CORES)))
    return assemble(res.results)
